# revision 3
# baseline (speedup 1.0000x reference)
"""Trainium2 Bass kernel for nn_NodeRNN (masked single-step LSTM over N nodes).

Strategy: pure data parallel over the node dim N across 8 cores. All per-node
tensors are staged FEATURE-MAJOR (transposed on host) so that every DMA is
contiguous and every matmul gets its contraction dim on partitions with no
on-device transposes. Outputs come back feature-major and are transposed back
on host.

Per 512-node tile (feature-major [features, nodes]):
  x.T   = [relu(W_pos @ xv.T + b_pos); relu(W_hid @ X.T + b_hid)]  (PE + ACT)
  gates = W_ih @ x.T + W_hh @ hv.T (+ biases via ACT)              (PE)
  i,f,o = sigmoid, g = tanh                                        (ACT)
  c_new = f*cv + i*g ; h_new = o*tanh(c_new)                       (DVE)
  hv/cv tiles conditionally overwritten where mask!=0              (DVE copy_predicated)
Matmuls run as float32r (1 col/cycle, ~1e-4 rel err) on f32 data.
"""
import sys

sys.path.insert(0, "/opt/trn_rl_repo")

import numpy as np

import concourse.bacc as bacc
import concourse.tile as tile
from concourse import mybir
from concourse.bass_utils import run_bass_kernel_spmd

f32 = mybir.dt.float32
f32r = mybir.dt.float32r
i32 = mybir.dt.int32
AF = mybir.ActivationFunctionType
ALU = mybir.AluOpType

N = 262144
NCORES = 8
NS = N // NCORES          # 32768 nodes per core
T = 512                   # nodes per tile
NT = NS // T              # 64 tiles per core
EMBED = 64
EDGE_H = 256
NODE_H = 128
XF = 2 * EDGE_H           # 512 concat(hvv, Hv) features

# const block layout: [128, CF] f32, free-dim offsets
CO_WHID = 0               # 4 chunks x 128 cols; cols 64:128 of chunk c = W_hid.T chunk
CO_WIH = 512              # W_ih.T [128, 512]
CO_WHH = 1024             # W_hh.T [128, 512]
CO_BX = 1536              # concat(b_pos, b_hid) [128, 1]
CO_BG = 1537              # (b_ih + b_hh) as [128, 4], col j = gate chunk j
CO_WP = 1541              # W_pos.T rows 0:2, [2, 64]
CO_ONES = 1605            # ones row at partition 32, [1, 128]
CF = 1760

GATE_FUNCS = [AF.Sigmoid, AF.Sigmoid, AF.Tanh, AF.Sigmoid]  # i, f, g, o

_cached = {}


def build_nc():
    nc = bacc.Bacc(target_bir_lowering=False)
    xt_d = nc.dram_tensor("xt", [XF, NS], f32r, kind="ExternalInput")
    hc_d = nc.dram_tensor("hc", [2 * NODE_H, NS], f32r, kind="ExternalInput")
    aux_d = nc.dram_tensor("aux", [33, NS], f32r, kind="ExternalInput")
    cst_d = nc.dram_tensor("cst", [128, CF], f32r, kind="ExternalInput")
    out_d = nc.dram_tensor("hc_out", [2 * NODE_H, NS], f32, kind="ExternalOutput")

    xt_v = xt_d[:].rearrange("(c p) n -> p c n", p=128)    # [128, 4, NS]
    hc_v = hc_d[:].rearrange("(c p) n -> p c n", p=128)    # [128, 2, NS]
    out_v = out_d[:].rearrange("(c p) n -> p c n", p=128)  # [128, 2, NS]

    with tile.TileContext(nc) as tc:
        with (
            tc.tile_pool(name="const", bufs=1) as cpool,
            tc.tile_pool(name="xt", bufs=3) as xtp,
            tc.tile_pool(name="hc", bufs=3) as hcp,
            tc.tile_pool(name="aux", bufs=3) as auxp,
            tc.tile_pool(name="xsb", bufs=2) as xsbp,
            tc.tile_pool(name="gact", bufs=6) as gactp,
            tc.tile_pool(name="tmp", bufs=2) as tmpp,
            tc.tile_pool(name="hcn", bufs=2) as hcnp,
            tc.tile_pool(name="ps_x", bufs=1, space="PSUM") as psx,
            tc.tile_pool(name="ps_g", bufs=6, space="PSUM") as psg,
            tc.tile_pool(name="ps_m", bufs=1, space="PSUM") as psm,
        ):
            cst = cpool.tile([128, CF], f32r)
            nc.sync.dma_start(cst[:], cst_d[:])

            # warmup matmul absorbs the cst DMA wait
            warm = psm.tile([1, 256], f32, tag="m")
            nc.tensor.matmul(warm[:], cst[32:33, CO_ONES:CO_ONES + 1],
                             cst[32:33, 0:256], start=True, stop=True)

            for t in range(NT):
                nsl = slice(t * T, (t + 1) * T)

                xt_t = xtp.tile([128, 4, T], f32r, tag="xt")
                nc.sync.dma_start(xt_t[:], xt_v[:, :, nsl])
                hc_t = hcp.tile([128, 2, T], f32r, tag="hc")
                nc.sync.dma_start(hc_t[:], hc_v[:, :, nsl])
                aux_t = auxp.tile([33, T], f32r, tag="aux")
                nc.sync.dma_start(aux_t[:], aux_d[:, nsl])

                # x.T psum: partitions 0:64 e_v, 64:128 a_v (padded lhsT)
                x_ps = psx.tile([128, T], f32, tag="x")
                for c in range(4):
                    nc.tensor.matmul(
                        x_ps[:, :], cst[:, CO_WHID + 128 * c:CO_WHID + 128 * (c + 1)],
                        xt_t[:, c, :], start=(c == 0), stop=False,
                        skip_group_check=True)
                nc.tensor.matmul(x_ps[0:64, :], cst[0:2, CO_WP:CO_WP + 64],
                                 aux_t[0:2, :], start=False, stop=True,
                                 skip_group_check=True)

                # mask broadcast [128, T]
                m_ps = psm.tile([128, T], f32, tag="m")
                nc.tensor.matmul(m_ps[:], cst[32:33, CO_ONES:CO_ONES + 128],
                                 aux_t[32:33, :], start=True, stop=True)

                # x = relu(x_ps + bias_x), rounded to f32r for the gate matmuls
                x_sb = xsbp.tile([128, T], f32r, tag="x_sb")
                nc.scalar.activation(x_sb[:], x_ps[:], AF.Relu,
                                     bias=cst[:, CO_BX:CO_BX + 1].bitcast(f32))

                # gates: per chunk j, g_ps_j = W_hh.T_j @ hv.T + W_ih.T_j @ x.T
                g_ps = []
                for j in range(4):
                    gp = psg.tile([128, T], f32, tag="g")
                    g_ps.append(gp)
                    nc.tensor.matmul(gp[:], cst[:, CO_WHH + 128 * j:CO_WHH + 128 * (j + 1)],
                                     hc_t[:, 0, :], start=True, stop=False)
                for j in range(4):
                    nc.tensor.matmul(g_ps[j][:], cst[:, CO_WIH + 128 * j:CO_WIH + 128 * (j + 1)],
                                     x_sb[:], start=False, stop=True)

                # gate nonlinearities with per-partition bias
                gact = []
                for j in range(4):
                    ga = gactp.tile([128, T], f32, tag="ga")
                    gact.append(ga)
                    nc.scalar.activation(ga[:], g_ps[j][:], GATE_FUNCS[j],
                                         bias=cst[:, CO_BG + j:CO_BG + j + 1].bitcast(f32))
                i_s, f_s, g_t, o_s = gact

                hcn = hcnp.tile([128, 2, T], f32, tag="hcn")
                t1 = tmpp.tile([128, T], f32, tag="t1")
                t2 = tmpp.tile([128, T], f32, tag="t2")
                th = tmpp.tile([128, T], f32, tag="th")
                cv_ap = hc_t[:, 1, :].bitcast(f32)
                hv_ap = hc_t[:, 0, :].bitcast(f32)
                # t1 = (f + 0) * cv ; t2 = (i + 0) * g ; c_new = (t1 + 0) + t2
                nc.vector.scalar_tensor_tensor(t1[:], f_s[:], 0.0, cv_ap, ALU.add, ALU.mult)
                nc.vector.scalar_tensor_tensor(t2[:], i_s[:], 0.0, g_t[:], ALU.add, ALU.mult)
                nc.vector.scalar_tensor_tensor(hcn[:, 1, :], t1[:], 0.0, t2[:], ALU.add, ALU.add)
                nc.scalar.activation(th[:], hcn[:, 1, :], AF.Tanh)
                # h_new = (o + 0) * tanh(c_new)
                nc.vector.scalar_tensor_tensor(hcn[:, 0, :], o_s[:], 0.0, th[:], ALU.add, ALU.mult)

                # m_ps broadcasts the INVERTED mask: overwrite h_new/c_new with
                # the old hv/cv on inactive rows, then store. (hc_t stays
                # read-only so its only producer is the f32r DMA.)
                nc.vector.copy_predicated(hcn[:, 0, :], m_ps[:].bitcast(i32), hv_ap)
                nc.vector.copy_predicated(hcn[:, 1, :], m_ps[:].bitcast(i32), cv_ap)
                nc.sync.dma_start(out_v[:, :, nsl], hcn[:])

    nc.finalize()
    return nc


def _stage_inputs(Hv_t, hvv_t, xv_t, hv_tm1, cv_tm1, ts_mask,
                  W_pos, b_pos, W_hid, b_hid, W_ih, b_ih, W_hh, b_hh):
    cst = np.zeros((128, CF), dtype=np.float32)
    whid_t = np.ascontiguousarray(W_hid.T)          # [512, 64]
    for c in range(4):
        cst[:, CO_WHID + 128 * c + 64:CO_WHID + 128 * (c + 1)] = whid_t[128 * c:128 * (c + 1)]
    cst[:, CO_WIH:CO_WIH + 512] = W_ih.T            # [128, 512]
    cst[:, CO_WHH:CO_WHH + 512] = W_hh.T
    cst[:, CO_BX] = np.concatenate([b_pos, b_hid])
    bg = b_ih + b_hh
    cst[:, CO_BG:CO_BG + 4] = bg.reshape(4, 128).T
    cst[0:2, CO_WP:CO_WP + 64] = W_pos.T
    cst[32, CO_ONES:CO_ONES + 128] = 1.0

    # inverted mask: 1.0 where the node is INACTIVE (keeps old state)
    maskf = (ts_mask[:, 0] != 1).astype(np.float32)

    in_maps = []
    for s in range(NCORES):
        sl = slice(s * NS, (s + 1) * NS)
        xt = np.empty((XF, NS), dtype=np.float32)
        xt[0:EDGE_H] = hvv_t[sl].T
        xt[EDGE_H:] = Hv_t[sl].T
        hc = np.empty((2 * NODE_H, NS), dtype=np.float32)
        hc[0:NODE_H] = hv_tm1[sl].T
        hc[NODE_H:] = cv_tm1[sl].T
        aux = np.zeros((33, NS), dtype=np.float32)
        aux[0:2] = xv_t[sl].T
        aux[32] = maskf[sl]
        in_maps.append(dict(xt=xt, hc=hc, aux=aux, cst=cst))
    return in_maps


def run(inputs, trace=False):
    """Stage, run on 8 cores, unstage. Returns ((hv_t, cv_t), BassKernelResults)."""
    inputs = {k: np.asarray(v) for k, v in inputs.items()}
    in_maps = _stage_inputs(**inputs)
    if "nc" not in _cached:
        _cached["nc"] = build_nc()
    res = run_bass_kernel_spmd(_cached["nc"], in_maps, core_ids=list(range(NCORES)),
                               trace=trace)
    hv_out = np.empty((N, NODE_H), dtype=np.float32)
    cv_out = np.empty((N, NODE_H), dtype=np.float32)
    for s in range(NCORES):
        sl = slice(s * NS, (s + 1) * NS)
        o = res.results[s]["hc_out"]
        hv_out[sl] = o[0:NODE_H].T
        cv_out[sl] = o[NODE_H:].T
    return (hv_out, cv_out), res


def kernel(**inputs):
    out, _ = run(inputs, trace=False)
    return out


# revision 5
# speedup vs baseline: 1.0248x; 1.0248x over previous
"""Trainium2 Bass kernel for nn_NodeRNN (masked single-step LSTM over N nodes).

Strategy: pure data parallel over the node dim N across 8 cores. All per-node
tensors are staged FEATURE-MAJOR (transposed on host) so that every DMA is
contiguous and every matmul gets its contraction dim on partitions with no
on-device transposes. Outputs come back feature-major and are transposed back
on host.

Per 512-node tile (feature-major [features, nodes]):
  x.T   = [relu(W_pos @ xv.T + b_pos); relu(W_hid @ X.T + b_hid)]  (PE + ACT)
  gates = W_ih @ x.T + W_hh @ hv.T (+ biases via ACT)              (PE)
  i,f,o = sigmoid, g = tanh                                        (ACT)
  c_new = f*cv + i*g ; h_new = o*tanh(c_new)                       (DVE)
  inactive rows get old hv/cv copied back over h_new/c_new         (DVE + GPSIMD mask bcast)
Matmuls run as float32r (1 col/cycle, ~1e-4 rel err) on f32 data.
Emission is software-pipelined (stage A of tile t+1 before stage B of tile t)
to keep the PE stream dense so the HAM clock stays warm.
"""
import sys

sys.path.insert(0, "/opt/trn_rl_repo")

import numpy as np

import concourse.bacc as bacc
import concourse.tile as tile
from concourse import mybir
from concourse.bass_utils import run_bass_kernel_spmd

f32 = mybir.dt.float32
f32r = mybir.dt.float32r
i32 = mybir.dt.int32
AF = mybir.ActivationFunctionType
ALU = mybir.AluOpType

N = 262144
NCORES = 8
NS = N // NCORES          # 32768 nodes per core
T = 512                   # nodes per tile
NT = NS // T              # 64 tiles per core
EMBED = 64
EDGE_H = 256
NODE_H = 128
XF = 2 * EDGE_H           # 512 concat(hvv, Hv) features

# const block layout: [128, CF] f32, free-dim offsets
CO_WHID = 0               # 4 chunks x 128 cols; cols 64:128 of chunk c = W_hid.T chunk
CO_WIH = 512              # W_ih.T [128, 512]
CO_WHH = 1024             # W_hh.T [128, 512]
CO_BX = 1536              # concat(b_pos, b_hid) [128, 1]
CO_BG = 1537              # (b_ih + b_hh) as [128, 4], col j = gate chunk j
CO_WP = 1541              # W_pos.T rows 0:2, [2, 64]
CF = 1632

GATE_FUNCS = [AF.Sigmoid, AF.Sigmoid, AF.Tanh, AF.Sigmoid]  # i, f, g, o

_cached = {}


def build_nc():
    nc = bacc.Bacc(target_bir_lowering=False)
    xt_d = nc.dram_tensor("xt", [XF, NS], f32r, kind="ExternalInput")
    hc_d = nc.dram_tensor("hc", [2 * NODE_H, NS], f32r, kind="ExternalInput")
    aux_d = nc.dram_tensor("aux", [2, NS], f32r, kind="ExternalInput")
    mk_d = nc.dram_tensor("mk", [1, NS], f32r, kind="ExternalInput")
    cst_d = nc.dram_tensor("cst", [128, CF], f32r, kind="ExternalInput")
    out_d = nc.dram_tensor("hc_out", [2 * NODE_H, NS], f32, kind="ExternalOutput")

    xt_v = xt_d[:].rearrange("(c p) n -> p c n", p=128)    # [128, 4, NS]
    hc_v = hc_d[:].rearrange("(c p) n -> p c n", p=128)    # [128, 2, NS]
    out_v = out_d[:].rearrange("(c p) n -> p c n", p=128)  # [128, 2, NS]

    with tile.TileContext(nc) as tc:
        with (
            tc.tile_pool(name="const", bufs=1) as cpool,
            tc.tile_pool(name="xt", bufs=3) as xtp,
            tc.tile_pool(name="hc", bufs=3) as hcp,
            tc.tile_pool(name="aux", bufs=3) as auxp,
            tc.tile_pool(name="xsb", bufs=2) as xsbp,
            tc.tile_pool(name="msk", bufs=2) as mskp,
            tc.tile_pool(name="gact", bufs=6) as gactp,
            tc.tile_pool(name="tmp", bufs=2) as tmpp,
            tc.tile_pool(name="hcn", bufs=2) as hcnp,
            tc.tile_pool(name="ps_x", bufs=2, space="PSUM") as psx,
            tc.tile_pool(name="ps_g", bufs=6, space="PSUM") as psg,
        ):
            cst = cpool.tile([128, CF], f32r)
            nc.sync.dma_start(cst[:], cst_d[:])

            # warmup matmul absorbs the cst DMA wait on the PE
            warm = psx.tile([64, 256], f32, tag="x")
            nc.tensor.matmul(warm[:], cst[0:2, CO_WP:CO_WP + 64],
                             cst[0:2, 0:256], start=True, stop=True)

            stash = {}

            def stage_a(t):
                nsl = slice(t * T, (t + 1) * T)
                xt_t = xtp.tile([128, 4, T], f32r, tag="xt")
                nc.sync.dma_start(xt_t[:], xt_v[:, :, nsl])
                hc_t = hcp.tile([128, 2, T], f32r, tag="hc")
                nc.sync.dma_start(hc_t[:], hc_v[:, :, nsl])
                aux_t = auxp.tile([2, T], f32r, tag="aux")
                nc.sync.dma_start(aux_t[:], aux_d[:, nsl])
                mk_t = auxp.tile([1, T], f32r, tag="mk")
                nc.sync.dma_start(mk_t[:], mk_d[:, nsl])

                # inverted-mask broadcast on the (otherwise idle) GPSIMD
                m_sb = mskp.tile([128, T], f32, tag="m")
                nc.gpsimd.partition_broadcast(m_sb[:], mk_t[:].bitcast(f32))

                # x.T psum: partitions 0:64 e_v, 64:128 a_v (zero-padded lhsT)
                x_ps = psx.tile([128, T], f32, tag="x")
                for c in range(4):
                    nc.tensor.matmul(
                        x_ps[:, :], cst[:, CO_WHID + 128 * c:CO_WHID + 128 * (c + 1)],
                        xt_t[:, c, :], start=(c == 0), stop=False,
                        skip_group_check=True)
                nc.tensor.matmul(x_ps[0:64, :], cst[0:2, CO_WP:CO_WP + 64],
                                 aux_t[0:2, :], start=False, stop=True,
                                 skip_group_check=True)

                # x = relu(x_ps + bias_x), rounded to f32r for the gate matmuls
                x_sb = xsbp.tile([128, T], f32r, tag="x_sb")
                nc.scalar.activation(x_sb[:], x_ps[:], AF.Relu,
                                     bias=cst[:, CO_BX:CO_BX + 1].bitcast(f32))
                stash[t] = (xt_t, hc_t, aux_t, m_sb, x_sb, nsl)

            def stage_b(t):
                xt_t, hc_t, aux_t, m_sb, x_sb, nsl = stash.pop(t)
                # gates: per chunk j, g_ps_j = W_hh.T_j @ hv.T + W_ih.T_j @ x.T
                g_ps = []
                for j in range(4):
                    gp = psg.tile([128, T], f32, tag="g")
                    g_ps.append(gp)
                    nc.tensor.matmul(gp[:], cst[:, CO_WHH + 128 * j:CO_WHH + 128 * (j + 1)],
                                     hc_t[:, 0, :], start=True, stop=False)
                for j in range(4):
                    nc.tensor.matmul(g_ps[j][:], cst[:, CO_WIH + 128 * j:CO_WIH + 128 * (j + 1)],
                                     x_sb[:], start=False, stop=True)

                # gate nonlinearities with per-partition bias
                gact = []
                for j in range(4):
                    ga = gactp.tile([128, T], f32, tag="ga")
                    gact.append(ga)
                    nc.scalar.activation(ga[:], g_ps[j][:], GATE_FUNCS[j],
                                         bias=cst[:, CO_BG + j:CO_BG + j + 1].bitcast(f32))
                i_s, f_s, g_t, o_s = gact

                hcn = hcnp.tile([128, 2, T], f32, tag="hcn")
                t1 = tmpp.tile([128, T], f32, tag="t1")
                t2 = tmpp.tile([128, T], f32, tag="t2")
                th = tmpp.tile([128, T], f32, tag="th")
                cv_ap = hc_t[:, 1, :].bitcast(f32)
                hv_ap = hc_t[:, 0, :].bitcast(f32)
                # t1 = (f + 0) * cv ; t2 = (i + 0) * g ; c_new = (t1 + 0) + t2
                nc.vector.scalar_tensor_tensor(t1[:], f_s[:], 0.0, cv_ap, ALU.add, ALU.mult)
                nc.vector.scalar_tensor_tensor(t2[:], i_s[:], 0.0, g_t[:], ALU.add, ALU.mult)
                nc.vector.scalar_tensor_tensor(hcn[:, 1, :], t1[:], 0.0, t2[:], ALU.add, ALU.add)
                nc.scalar.activation(th[:], hcn[:, 1, :], AF.Tanh)
                # h_new = (o + 0) * tanh(c_new)
                nc.vector.scalar_tensor_tensor(hcn[:, 0, :], o_s[:], 0.0, th[:], ALU.add, ALU.mult)

                # m_sb broadcasts the INVERTED mask: overwrite h_new/c_new with
                # the old hv/cv on inactive rows, then store. (hc_t stays
                # read-only so its only producer is the f32r DMA.)
                nc.vector.copy_predicated(hcn[:, 0, :], m_sb[:].bitcast(i32), hv_ap)
                nc.vector.copy_predicated(hcn[:, 1, :], m_sb[:].bitcast(i32), cv_ap)
                nc.sync.dma_start(out_v[:, :, nsl], hcn[:])

            for t in range(NT + 1):
                if t < NT:
                    stage_a(t)
                if t >= 1:
                    stage_b(t - 1)

    nc.finalize()
    return nc


def _stage_inputs(Hv_t, hvv_t, xv_t, hv_tm1, cv_tm1, ts_mask,
                  W_pos, b_pos, W_hid, b_hid, W_ih, b_ih, W_hh, b_hh):
    cst = np.zeros((128, CF), dtype=np.float32)
    whid_t = np.ascontiguousarray(W_hid.T)          # [512, 64]
    for c in range(4):
        cst[:, CO_WHID + 128 * c + 64:CO_WHID + 128 * (c + 1)] = whid_t[128 * c:128 * (c + 1)]
    cst[:, CO_WIH:CO_WIH + 512] = W_ih.T            # [128, 512]
    cst[:, CO_WHH:CO_WHH + 512] = W_hh.T
    cst[:, CO_BX] = np.concatenate([b_pos, b_hid])
    bg = b_ih + b_hh
    cst[:, CO_BG:CO_BG + 4] = bg.reshape(4, 128).T
    cst[0:2, CO_WP:CO_WP + 64] = W_pos.T

    # inverted mask: 1.0 where the node is INACTIVE (keeps old state)
    maskf = (ts_mask[:, 0] != 1).astype(np.float32)

    in_maps = []
    for s in range(NCORES):
        sl = slice(s * NS, (s + 1) * NS)
        xt = np.empty((XF, NS), dtype=np.float32)
        xt[0:EDGE_H] = hvv_t[sl].T
        xt[EDGE_H:] = Hv_t[sl].T
        hc = np.empty((2 * NODE_H, NS), dtype=np.float32)
        hc[0:NODE_H] = hv_tm1[sl].T
        hc[NODE_H:] = cv_tm1[sl].T
        aux = np.ascontiguousarray(xv_t[sl].T)
        mk = maskf[sl].reshape(1, NS)
        in_maps.append(dict(xt=xt, hc=hc, aux=aux, mk=mk, cst=cst))
    return in_maps


def run(inputs, trace=False):
    """Stage, run on 8 cores, unstage. Returns ((hv_t, cv_t), BassKernelResults)."""
    inputs = {k: np.asarray(v) for k, v in inputs.items()}
    in_maps = _stage_inputs(**inputs)
    if "nc" not in _cached:
        _cached["nc"] = build_nc()
    res = run_bass_kernel_spmd(_cached["nc"], in_maps, core_ids=list(range(NCORES)),
                               trace=trace)
    hv_out = np.empty((N, NODE_H), dtype=np.float32)
    cv_out = np.empty((N, NODE_H), dtype=np.float32)
    for s in range(NCORES):
        sl = slice(s * NS, (s + 1) * NS)
        o = res.results[s]["hc_out"]
        hv_out[sl] = o[0:NODE_H].T
        cv_out[sl] = o[NODE_H:].T
    return (hv_out, cv_out), res


def kernel(**inputs):
    out, _ = run(inputs, trace=False)
    return out
